# revision 1
# baseline (speedup 1.0000x reference)
"""Channel-attention (per-head [64,64] score matrix) Trainium2 Bass kernel.

Math (per batch b of 16):
    qkv = x @ w_qkv                 # x [4096, 256], w_qkv [256, 1536]
    q,k,v = split(qkv); per head h (8 heads x 64 dim):
    sim_h = (q_h * 8^-1)^T @ k_h    # [64, 64]   (contracts spatial d=4096)
    attn_h = softmax(sim_h, axis=-1)
    out_h = v_h @ attn_h^T          # [4096, 64]
    y = concat(out_h) @ w_out + b_out

Distribution: data-parallel over batch — 8 cores x 2 batches each; weights
replicated; no collectives. The host pre-transposes x to [C, d] per batch so
every device matmul streams with large free dims, pre-folds the 1/8 q-scale
into w_q, pre-converts inputs to fp16 (all matmuls run fp16 x fp16 with fp32
PSUM accumulation; end-to-end rel-l2 ~1.6e-3 vs fp64 oracle), and adds the
output bias on the host (so y can DMA straight out of PSUM).

Device dataflow per batch (phases ordered so V-phase matmuls hide the
softmax latency on PE):
  QK:   q,k [d-chunk 128, 512each] (lhsT = xT d-chunk, rhs = w_qk cols, N=512)
  B:    sim[p] [128,128] per head-pair accumulates over 32 d-chunks
  V:    vT[m,d] = w_v.T @ xT       (lhsT = w_v chunk, rhs = xT d-cols, N=512)
  soft: rowmax (negated) -> exp(sim - max) with accum_out row-sums ->
        recip -> scale e rows by 1/s (so C1's PSUM drain is a plain copy)
  T:    PE-transpose e_p -> eT_p (C1's stationary operand)
  C1:   outT[i,d] = eT_h @ vT_h, two heads per PE pass (row/col split)
  C2:   y[d,c] = outT.T @ w_out, DMA'd to HBM directly from PSUM (fp32)
"""

import numpy as np

import concourse.bass as bass
import concourse.mybir as mybir
from concourse.bass_utils import run_bass_kernel_spmd
from concourse.masks import make_identity
from concourse.tile import TileContext


def _split_multi_waits(nc, limit=1):
    """Post-pass: the walrus build in this container rejects instructions
    carrying more than `limit` sync-waits ("Too many sync wait commands" in
    setupSyncWait). Tile attaches up to 3. Hoist the extras onto same-engine
    NoOp instructions inserted immediately before the owner — the engine
    sequencer executes them in order, so the ordering semantics are
    identical (single-wait instructions are what the rest of the Tile
    output uses, and those compile)."""
    drain_engines = [
        mybir.EngineType.PE,
        mybir.EngineType.DVE,
        mybir.EngineType.Activation,
        mybir.EngineType.Pool,
        mybir.EngineType.SP,
    ]
    n_split = 0
    for f in nc.m.functions:
        for blk in f.blocks:
            il = blk.instructions
            i = 0
            while i < len(il):
                inst = il[i]
                si = inst.sync_info
                waits = list(si.on_wait) if si is not None else []
                if len(waits) > limit:
                    si.on_wait = waits[:limit]
                    # The kernel-tail drain aggregates one wait per logical
                    # processor; those can wait in parallel across engines
                    # (the all-engine barrier that follows orders them before
                    # the semaphore clears). Mid-program instructions keep
                    # their extras on their own engine to preserve ordering.
                    is_drain = type(inst).__name__ == "InstDrain"
                    for k, w in enumerate(waits[limit:]):
                        nop = mybir.InstNoOp(
                            name=f"I-waitsplit-{n_split}", ins=[], outs=[]
                        )
                        n_split += 1
                        nop.engine = (
                            drain_engines[k % len(drain_engines)]
                            if is_drain else inst.engine
                        )
                        nop.sync_info = mybir.SyncInfo(on_wait=[w], on_update=[])
                        il.insert(i, nop)
                        i += 1
                i += 1
    return nc


N_CORES = 8
BATCH = 16
BPC = BATCH // N_CORES  # batches per core
D = 4096  # spatial (64*64)
C = 256   # channels
HID = 512
HEADS = 8
DH = 64

F32 = mybir.dt.float32
F16 = mybir.dt.float16

_CACHE = {}


def _build():
    nc = bass.Bass()
    xT_d = nc.declare_dram_parameter("xT", [BPC, C, D], F16, isOutput=False)
    wqkv_d = nc.declare_dram_parameter("w_qkv", [C, 3 * HID], F16, isOutput=False)
    wout_d = nc.declare_dram_parameter("w_out_r", [128, 4, C], F16, isOutput=False)
    y_d = nc.declare_dram_parameter("y", [BPC, D, C], F32, isOutput=True)

    with TileContext(nc) as tc:
        with (
            tc.tile_pool(name="consts", bufs=1) as consts,
            tc.tile_pool(name="xt", bufs=2) as xt_pool,
            tc.tile_pool(name="vt", bufs=8) as vt_pool,
            tc.tile_pool(name="qk", bufs=6) as qk_pool,
            tc.tile_pool(name="eP", bufs=8) as e_pool,
            tc.tile_pool(name="stat", bufs=6) as stat_pool,
            tc.tile_pool(name="ot", bufs=12) as ot_pool,
            tc.tile_pool(name="ysb", bufs=8) as y_pool,
            tc.tile_pool(name="mm", bufs=6, space="PSUM") as mm_pool,
            tc.tile_pool(name="simp", bufs=2, space="PSUM") as sim_pool,
        ):
            # ---- constants ----
            # w_qkv split loads ordered by first use: w_q, then w_k, then
            # w_v / w_out (V and C2 run much later).
            w_sb = []
            for ci in range(2):
                w_t = consts.tile([128, 3 * HID], F16, name=f"w{ci}")
                w_sb.append(w_t)
            for ci in range(2):
                nc.sync.dma_start(
                    out=w_sb[ci][:, 0:HID],
                    in_=wqkv_d[ci * 128:(ci + 1) * 128, 0:HID],
                )
            wo_sb = consts.tile([128, 4, C], F16, name="wo")
            ident = consts.tile([128, 128], F32, name="ident")
            make_identity(nc, ident)

            for b in range(BPC):
                # ---- load xT (chunked so the first QK matmuls start early) --
                xt = []
                for ci in range(2):
                    x_t = xt_pool.tile([128, D], F16, name=f"xt{ci}", tag="xt")
                    xt.append(x_t)
                # first 512 cols arrive alone so QK d1=0..3 can start
                # early; w_k loads are interleaved after them (the k matmuls
                # trail the q matmuls by the pipeline skew anyway)
                chunks = [(0, 512)] + [(lo, lo + 896) for lo in range(512, D, 896)]
                for ki, (lo, hi) in enumerate(chunks):
                    hi = min(hi, D)
                    for ci in range(2):
                        nc.sync.dma_start(
                            out=xt[ci][:, lo:hi],
                            in_=xT_d[b, ci * 128:(ci + 1) * 128, lo:hi],
                        )
                    if b == 0 and ki == 0:
                        for ci in range(2):
                            nc.sync.dma_start(
                                out=w_sb[ci][:, HID:2 * HID],
                                in_=wqkv_d[ci * 128:(ci + 1) * 128, HID:2 * HID],
                            )

                # ---- phase QK + B ----
                # sim[p]: one PSUM bank per accumulation group (start=True
                # zeroes a whole 2KB zero-region per written partition, so
                # groups must not share a bank). Tile p = head pair
                # (2p, 2p+1): rows i (head 2p at 0:64, 2p+1 at 64:128),
                # cols j likewise; diag 64x64 blocks are the per-head sims.
                # sim_all [128, 256]: ONE psum bank holds all 8 per-head
                # accumulators — pair p at cols p*64:+64, head 2p at rows
                # 0:64, head 2p+1 at rows 64:128. The bank is zeroed by an
                # explicit memset and every matmul uses start=False
                # (accumulate) — order-independent, so the scheduler may
                # interleave the groups freely.
                sim_all = sim_pool.tile([128, 256], F32, name="sim_all", tag="simp")
                nc.vector.memset(sim_all, 0.0)
                def emit_b(qk_tile, d1):
                    # sim matmuls for the qk tile of iteration d1 (emitted one
                    # iteration late so the PSUM->SBUF copy latency hides
                    # under the next iteration's qk matmuls)
                    for p in range(4):
                        for par in range(2):
                            q_lo = p * 128 + par * 64
                            nc.tensor.matmul(
                                sim_all[par * 64:(par + 1) * 64, p * 64:(p + 1) * 64],
                                lhsT=qk_tile[:, q_lo:q_lo + 64],
                                rhs=qk_tile[:, 512 + q_lo:512 + q_lo + 64],
                                start=False,
                                stop=(d1 == 31),
                                skip_group_check=True,
                            )

                prev = None
                for d1 in range(32):
                    qps = mm_pool.tile([128, 512], F32, name="qps", tag="mm")
                    kps = mm_pool.tile([128, 512], F32, name="kps", tag="mm")
                    for ci in range(2):
                        nc.tensor.matmul(
                            qps,
                            lhsT=xt[ci][:, d1 * 128:(d1 + 1) * 128],
                            rhs=w_sb[ci][:, 0:HID],
                            start=(ci == 0),
                            stop=(ci == 1),
                        )
                    for ci in range(2):
                        nc.tensor.matmul(
                            kps,
                            lhsT=xt[ci][:, d1 * 128:(d1 + 1) * 128],
                            rhs=w_sb[ci][:, HID:2 * HID],
                            start=(ci == 0),
                            stop=(ci == 1),
                        )
                    qk = qk_pool.tile([128, 1024], F16, name="qk", tag="qk")
                    nc.any.tensor_copy(qk[:, 0:512], qps)
                    nc.any.tensor_copy(qk[:, 512:1024], kps)
                    if prev is not None:
                        emit_b(*prev)
                    prev = (qk, d1)

                # ---- phase V (PE work that hides softmax latency) ----
                # d5-outer so vt[0..3] become ready column-range by
                # column-range — C1's d5 loop can start at d5=0 early. The
                # first d5 iteration is emitted BEFORE the last deferred B
                # matmuls so the scheduler has PE work to cover the final
                # qk copy's latency.
                if b == 0:
                    # deferred weight loads (not needed until now)
                    for ci in range(2):
                        nc.sync.dma_start(
                            out=w_sb[ci][:, 2 * HID:3 * HID],
                            in_=wqkv_d[ci * 128:(ci + 1) * 128, 2 * HID:3 * HID],
                        )
                    nc.sync.dma_start(out=wo_sb, in_=wout_d[:, :, :])
                vt = []
                for m in range(4):
                    v_t = vt_pool.tile([128, D], F16, name=f"vt{m}", tag="vt")
                    vt.append(v_t)

                def emit_v(d5):
                    for m in range(4):
                        wv_lo = 2 * HID + m * 128
                        vps = mm_pool.tile([128, 512], F32, name="vps", tag="mm")
                        for ci in range(2):
                            nc.tensor.matmul(
                                vps,
                                lhsT=w_sb[ci][:, wv_lo:wv_lo + 128],
                                rhs=xt[ci][:, d5 * 512:(d5 + 1) * 512],
                                start=(ci == 0),
                                stop=(ci == 1),
                            )
                        nc.any.tensor_copy(vt[m][:, d5 * 512:(d5 + 1) * 512], vps)

                emit_b(*prev)
                for d5 in range(8):
                    emit_v(d5)

                # ---- softmax (DVE/ACT; overlaps V on PE) ----
                # head h: pair p=h//2, par=h%2; diag block of sim[p] at
                # rows/cols par*64:+64.
                m_t = stat_pool.tile([128, 4], F32, name="m_t", tag="stat")
                s_t = stat_pool.tile([128, 4], F32, name="s_t", tag="stat")
                r_t = stat_pool.tile([128, 4], F32, name="r_t", tag="stat")
                e_tiles = []
                for p in range(4):
                    e_p = e_pool.tile([128, 128], F32, name=f"e{p}", tag="e")
                    nc.gpsimd.memset(e_p, 0.0)
                    e_tiles.append(e_p)
                for h in range(HEADS):
                    par, p = h % 2, h // 2
                    rows = slice(par * 64, par * 64 + 64)
                    nc.vector.reduce_max(
                        out=m_t[rows, p:p + 1],
                        in_=sim_all[rows, p * 64:(p + 1) * 64],
                        axis=mybir.AxisListType.X,
                        negate=True,
                    )
                for h in range(HEADS):
                    par, p = h % 2, h // 2
                    rows = slice(par * 64, par * 64 + 64)
                    nc.scalar.activation(
                        out=e_tiles[p][rows, par * 64:par * 64 + 64],
                        in_=sim_all[rows, p * 64:(p + 1) * 64],
                        func=mybir.ActivationFunctionType.Exp,
                        bias=m_t[rows, p:p + 1],
                        scale=1.0,
                        accum_out=s_t[rows, p:p + 1],
                    )
                nc.vector.reciprocal(r_t, s_t)
                # attn = e / s: fold 1/s into e rows now (tiny [128,128]
                # tiles) instead of scaling every [128,512] C1 output.
                for p in range(4):
                    nc.vector.tensor_scalar_mul(
                        e_tiles[p], e_tiles[p], r_t[:, p:p + 1]
                    )

                # ---- transpose e -> eT (PE) ----
                eT_tiles = []
                for p in range(4):
                    etps = mm_pool.tile([128, 128], F32, name="etps", tag="mm")
                    nc.tensor.transpose(etps, e_tiles[p], ident)
                    eT_s = e_pool.tile([128, 128], F16, name=f"eT{p}", tag="eT")
                    nc.any.tensor_copy(eT_s, etps)
                    eT_tiles.append(eT_s)

                # ---- phase C: attention-apply + output projection ----
                def emit_c2(ot_tiles, d5):
                    # C2 for d5's ot tiles (emitted one d5 late so the ot
                    # copy latency hides under the next d5's C1 matmuls)
                    for d1 in range(4):
                        yps = mm_pool.tile([128, C], F32, name="yps", tag="mm")
                        for p4 in range(4):
                            nc.tensor.matmul(
                                yps,
                                lhsT=ot_tiles[p4][:, d1 * 128:(d1 + 1) * 128],
                                rhs=wo_sb[:, p4, :],
                                start=(p4 == 0),
                                stop=(p4 == 3),
                            )
                        ysb = y_pool.tile([128, C], F32, name="ysb", tag="ysb")
                        nc.any.tensor_copy(ysb, yps)
                        d_lo = d5 * 512 + d1 * 128
                        nc.sync.dma_start(out=y_d[b, d_lo:d_lo + 128, :], in_=ysb)

                prev_c = None
                for d5 in range(8):
                    ot_tiles = []
                    for p in range(4):
                        c1ps = mm_pool.tile([128, 512], F32, name="c1ps", tag="mm")
                        # eT_p is exactly block-diagonal (off-diag blocks are
                        # memset zeros), so one full-array K=128 matmul
                        # computes both heads: rows 0:64 of eT only meet
                        # vt rows 0:64 (head 2p), rows 64:128 only head 2p+1.
                        nc.tensor.matmul(
                            c1ps,
                            lhsT=eT_tiles[p],
                            rhs=vt[p][:, d5 * 512:(d5 + 1) * 512],
                            start=True,
                            stop=True,
                        )
                        ot = ot_pool.tile([128, 512], F16, name=f"ot{p}", tag="ot")
                        nc.any.tensor_copy(ot, c1ps)
                        ot_tiles.append(ot)
                    if prev_c is not None:
                        emit_c2(*prev_c)
                    prev_c = (ot_tiles, d5)
                emit_c2(*prev_c)
    return _split_multi_waits(nc)


def _get_nc():
    if "nc" not in _CACHE:
        _CACHE["nc"] = _build()
    return _CACHE["nc"]


def kernel(x, w_qkv, w_out, b_out, **kw):
    x = np.asarray(x, dtype=np.float32)
    w_qkv = np.asarray(w_qkv, dtype=np.float32)
    w_out = np.asarray(w_out, dtype=np.float32)
    b_out = np.asarray(b_out, dtype=np.float32)

    # fold q-scale into w_q (exact: power-of-two scale), then fp16-quantize
    w_qkv_s = w_qkv.copy()
    w_qkv_s[:, :HID] *= DH ** (-0.5)
    w_qkv_s = np.ascontiguousarray(w_qkv_s.astype(np.float16))
    # w_out [512, 256] -> [128, 4, 256] with [p, t, c] = w_out[t*128+p, c]
    w_out_r = np.ascontiguousarray(
        w_out.reshape(4, 128, C).transpose(1, 0, 2).astype(np.float16)
    )

    x4 = x.reshape(BATCH, D, C).astype(np.float16)
    in_maps = []
    for core in range(N_CORES):
        xs = np.ascontiguousarray(
            x4[core * BPC:(core + 1) * BPC].transpose(0, 2, 1)
        )  # [BPC, C, D] fp16
        in_maps.append({"xT": xs, "w_qkv": w_qkv_s, "w_out_r": w_out_r})

    nc = _get_nc()
    res = run_bass_kernel_spmd(nc, in_maps, core_ids=list(range(N_CORES)), **kw)
    y = np.concatenate([r["y"] for r in res.results], axis=0)  # [16, 4096, 256]
    y += b_out  # bias on host (broadcast over last axis)
    return y.reshape(BATCH, 64, 64, C)



# revision 22
# speedup vs baseline: 3.4624x; 3.4624x over previous
"""Channel-attention (per-head [64,64] score matrix) Trainium2 Bass kernel.

Algebraic restructuring vs the direct q/k/v formulation: since the score
matrix contracts the full spatial axis, attention only needs the Gram matrix
    G = x^T x                       # [256, 256], 268M MACs
    sim_h = (w_q_h/8)^T G w_k_h     # [64, 64] per head (t = G @ w_k first)
    attn_h = softmax(sim_h)
    W2 = sum_h w_v_h attn_h^T w_out_h   # [256, 256] fused output operator
    y = x @ W2 (+ b_out on host)    # 268M MACs
Total ~620M MACs/batch vs 2.4G for the direct path (~4x less PE work).

Distribution: data-parallel over batch - 8 cores x 2 batches each, weights
replicated, no collectives. Host sends x in BOTH layouts (natural [d,C] for
G, transposed [C,d] for y) because the PE contracts only the partition dim;
everything is fp16 with fp32 PSUM accumulation. y returns fp16; bias-add and
fp32 upcast happen on the host.

Per-core schedule (two batches pipelined so softmax/DVE latency hides under
the other batch's PE phases; PE gaps kept < ~1us to avoid p-state re-ramp):
  G0 | G1[0:8] t0 G1[8:16] sim0 G1[16:64] | aT0 t1 uT0 W20 sim1 |
  y0[0:24] aT1 y0[24:32] uT1 W21 | y1
PSUM accumulators that share a bank use an explicit memset + start=False
(order-independent) exactly like the proven sim_all pattern.
"""

import numpy as np

import concourse.bass as bass
import concourse.mybir as mybir
from concourse.bass_utils import run_bass_kernel_spmd
from concourse.masks import make_identity
from concourse.tile import TileContext


def _split_multi_waits(nc, limit=1):
    """Post-pass: the walrus build in this container rejects instructions
    carrying more than `limit` sync-waits ("Too many sync wait commands" in
    setupSyncWait). Tile attaches up to 3. Hoist the extras onto same-engine
    NoOp instructions inserted immediately before the owner - the engine
    sequencer executes them in order, so the ordering semantics are
    identical."""
    drain_engines = [
        mybir.EngineType.PE,
        mybir.EngineType.DVE,
        mybir.EngineType.Activation,
        mybir.EngineType.Pool,
        mybir.EngineType.SP,
    ]
    n_split = 0
    for f in nc.m.functions:
        for blk in f.blocks:
            il = blk.instructions
            i = 0
            while i < len(il):
                inst = il[i]
                si = inst.sync_info
                waits = list(si.on_wait) if si is not None else []
                if len(waits) > limit:
                    si.on_wait = waits[:limit]
                    is_drain = type(inst).__name__ == "InstDrain"
                    for k, w in enumerate(waits[limit:]):
                        nop = mybir.InstNoOp(
                            name=f"I-waitsplit-{n_split}", ins=[], outs=[]
                        )
                        n_split += 1
                        nop.engine = (
                            drain_engines[k % len(drain_engines)]
                            if is_drain else inst.engine
                        )
                        nop.sync_info = mybir.SyncInfo(on_wait=[w], on_update=[])
                        il.insert(i, nop)
                        i += 1
                i += 1
    return nc


N_CORES = 8
BATCH = 16
BPC = BATCH // N_CORES  # batches per core
D = 4096   # spatial (64*64)
C = 256    # channels
HID = 512
HEADS = 8
DH = 64
NK = 32    # d-chunks of 128

F32 = mybir.dt.float32
F16 = mybir.dt.float16
BF16 = mybir.dt.bfloat16

_CACHE = {}


def _build():
    nc = bass.Bass()
    xn_d = nc.declare_dram_parameter("xN", [BPC, 128, NK, C], F16, isOutput=False)
    xt_d = nc.declare_dram_parameter("xT", [BPC, 2, 128, D], F16, isOutput=False)
    wqk_d = nc.declare_dram_parameter("wqk", [2, 128, 1024], F16, isOutput=False)
    wvt_d = nc.declare_dram_parameter("wvt", [64, HEADS, C], F16, isOutput=False)
    wo_d = nc.declare_dram_parameter("wo", [128, 4, C], F16, isOutput=False)
    y_d = nc.declare_dram_parameter("y", [BPC, 128, NK, C], F16, isOutput=True)

    with TileContext(nc) as tc:
        with (
            tc.tile_pool(name="consts", bufs=1) as consts,
            tc.tile_pool(name="xn", bufs=2) as xn_pool,
            tc.tile_pool(name="xt", bufs=4) as xt_pool,
            tc.tile_pool(name="small", bufs=12) as sm_pool,
            tc.tile_pool(name="ysb", bufs=8) as y_sb_pool,
            tc.tile_pool(name="gps", bufs=2, space="PSUM") as g_pool,
            tc.tile_pool(name="big", bufs=3, space="PSUM") as big_pool,
            tc.tile_pool(name="simp", bufs=1, space="PSUM") as sim_pool,
            tc.tile_pool(name="atut", bufs=2, space="PSUM") as atut_pool,
        ):
            # ---- constant tiles ----
            wqk_sb = [consts.tile([128, 1024], F16, name=f"wqk{ci}") for ci in (0, 1)]
            wvt_sb = consts.tile([64, HEADS, C], F16, name="wvt")
            wo_sb = consts.tile([128, 4, C], F16, name="wo")
            ident32 = consts.tile([128, 128], F32, name="ident32")
            make_identity(nc, ident32)
            identh = consts.tile([128, 128], F16, name="identh")
            make_identity(nc, identh)

            # per-batch SBUF tiles
            xn_t = [xn_pool.tile([128, NK, C], F16, name=f"xn{b}", tag="xn")
                    for b in (0, 1)]
            xt_t = [[xt_pool.tile([128, D], F16, name=f"xt{b}{ci}", tag="xt")
                     for ci in (0, 1)] for b in (0, 1)]
            g_sb = [sm_pool.tile([128, 512], F16, name=f"gsb{b}", tag="gsb")
                    for b in (0, 1)]
            tq_sb = [[sm_pool.tile([128, 512], F16, name=f"tqsb{b}{cc}", tag="tqsb")
                      for cc in (0, 1)] for b in (0, 1)]
            s_t = [sm_pool.tile([128, 4], F32, name=f"s{b}", tag="stat") for b in (0, 1)]
            r_t = [sm_pool.tile([128, 4], F32, name=f"r{b}", tag="stat") for b in (0, 1)]
            m_t = [sm_pool.tile([128, 4], F32, name=f"m{b}", tag="stat")
                   for b in (0, 1)]
            apair = [sm_pool.tile([128, 4, 64], F32, name=f"ap{b}", tag="ap")
                     for b in (0, 1)]
            at_sb = [sm_pool.tile([64, 4, 128], F16, name=f"at{b}", tag="at")
                     for b in (0, 1)]
            ut_sb = [sm_pool.tile([128, 4, C], F16, name=f"ut{b}", tag="ut")
                     for b in (0, 1)]
            w2_sb = [sm_pool.tile([128, 2, C], F16, name=f"w2{b}", tag="w2")
                     for b in (0, 1)]

            g_ps = [g_pool.tile([128, 512], F32, name=f"gps{b}", tag="g")
                    for b in (0, 1)]
            w2_ps = [None, None]

            def copy2(i, out, in_):
                # alternate DVE tensor_copy / ACT activation-copy
                if i % 2 == 0:
                    nc.vector.tensor_copy(out, in_)
                else:
                    nc.scalar.copy(out, in_)

            # ---------------- emission helpers ----------------
            def emit_xn_dma(b):
                lo = 0
                for gsz in (2, 3, 4, 4, 4, 5, 5, 5):
                    hi = lo + gsz
                    nc.sync.dma_start(
                        out=xn_t[b][:, lo:hi, :], in_=xn_d[b, :, lo:hi, :]
                    )
                    lo = hi

            def emit_xt_dma(b):
                for ci in (0, 1):
                    for half in (0, 1):
                        nc.sync.dma_start(
                            out=xt_t[b][ci][:, half * 2048:(half + 1) * 2048],
                            in_=xt_d[b, ci, :, half * 2048:(half + 1) * 2048],
                        )

            def emit_g(b, k0, k1):
                # G symmetric: only G00, G01, G11 accumulate (G10 = G01^T is
                # reconstructed by a PE transpose afterwards). First write
                # zeroes the whole bank.
                for k in range(k0, k1):
                    for qi, (ca, cb) in enumerate(((0, 0), (0, 1), (1, 1))):
                        nc.tensor.matmul(
                            g_ps[b][:, qi * 128:(qi + 1) * 128],
                            lhsT=xn_t[b][:, k, ca * 128:(ca + 1) * 128],
                            rhs=xn_t[b][:, k, cb * 128:(cb + 1) * 128],
                            start=(k == 0 and qi == 0),
                            stop=(k == NK - 1 and qi == 2),
                            skip_group_check=True,
                        )

            def emit_g_copies(b, g10_ps):
                # drain the three computed quarters, then rebuild G10 = G01^T
                # on the PE (consumed last by the tq matmuls)
                copy2(0, g_sb[b][:, 128:256], g_ps[b][:, 128:256])
                copy2(1, g_sb[b][:, 0:128], g_ps[b][:, 0:128])
                nc.tensor.matmul(
                    g10_ps, lhsT=g_sb[b][:, 128:256], rhs=identh,
                    is_transpose=True, start=True, stop=True,
                    skip_group_check=True,
                )
                copy2(1, g_sb[b][:, 256:384], g_ps[b][:, 256:384])
                copy2(0, g_sb[b][:, 384:512], g10_ps)

            TQ_Q = {(0, 0): 0, (0, 1): 3, (1, 0): 1, (1, 1): 2}

            def emit_tq(b, tq_ps):
                # t = G @ w_k; order so the reconstructed Q10 is needed last
                for cc in (1, 0):
                    for ci2 in (0, 1):
                        q = TQ_Q[(cc, ci2)]
                        nc.tensor.matmul(
                            tq_ps[cc],
                            lhsT=g_sb[b][:, q * 128:(q + 1) * 128],
                            rhs=wqk_sb[ci2][:, 512:1024],
                            start=(ci2 == 0), stop=(ci2 == 1),
                        )

            def emit_tq_copies(b, tq_ps):
                for cc in (0, 1):
                    copy2(cc, tq_sb[b][cc], tq_ps[cc])

            def emit_simt(b, simt_ps):
                # sim_h[i, j] = w_q_h^T (G w_k_h): pair p in col block p,
                # head parity in row halves (i on partitions for the softmax)
                for h in range(HEADS):
                    p, par = h // 2, h % 2
                    for cc in (0, 1):
                        nc.tensor.matmul(
                            simt_ps[par * 64:par * 64 + 64, p * 64:(p + 1) * 64],
                            lhsT=wqk_sb[cc][:, h * 64:h * 64 + 64],
                            rhs=tq_sb[b][cc][:, h * 64:h * 64 + 64],
                            start=(h < 2 and cc == 0), stop=(cc == 1),
                            skip_group_check=True,
                        )

            def emit_exp(b, simt_ps):
                # stable softmax numerator: one fused row-max (negated) then
                # per-pair biased exp; e <= 1 afterwards
                nc.vector.reduce_max(
                    out=m_t[b][:, :],
                    in_=simt_ps[:, :].rearrange("p (a b) -> p a b", a=4),
                    axis=mybir.AxisListType.X,
                    negate=True,
                )
                for p in range(4):
                    nc.scalar.activation(
                        out=apair[b][:, p, :],
                        in_=simt_ps[:, p * 64:(p + 1) * 64],
                        func=mybir.ActivationFunctionType.Exp,
                        bias=m_t[b][:, p:p + 1],
                        scale=1.0,
                    )

            def emit_eprime(b, e_ps):
                # transpose e to [j, (p, i-stacked)] as the uT stationary
                for p in range(4):
                    nc.tensor.matmul(
                        e_ps[:, p * 128:(p + 1) * 128],
                        lhsT=apair[b][:, p, :],
                        rhs=ident32,
                        is_transpose=True,
                        start=(p == 0), stop=(p == 3),
                        skip_group_check=True,
                    )

            def emit_at_copy(b, e_ps):
                copy2(b, at_sb[b][:, :, :], e_ps[:, :])

            def emit_sums(b):
                nc.vector.reduce_sum(
                    out=s_t[b][:, :],
                    in_=apair[b][:, :, :],
                    axis=mybir.AxisListType.X,
                )
                nc.vector.reciprocal(r_t[b], s_t[b])

            def emit_ut(b, ut_ps, pp):
                # raw (unnormalized) uT: lhsT = e_h^T slice, rhs = w_v^T rows
                for dp in (0, 1):
                    p = 2 * pp + dp
                    for par in (0, 1):
                        h = 2 * p + par
                        nc.tensor.matmul(
                            ut_ps[par * 64:par * 64 + 64, dp * 256:(dp + 1) * 256],
                            lhsT=at_sb[b][:, p, par * 64:par * 64 + 64],
                            rhs=wvt_sb[:, h, :],
                            start=(dp == 0), stop=(dp == 1),
                            skip_group_check=True,
                        )

            def emit_ut_copies(b, ut_ps, pp):
                # fold the softmax normalizer in during the PSUM drain
                for dp in (0, 1):
                    p = 2 * pp + dp
                    if dp == 0:
                        nc.vector.tensor_scalar_mul(
                            ut_sb[b][:, p, :], ut_ps[:, dp * 256:(dp + 1) * 256],
                            r_t[b][:, p:p + 1],
                        )
                    else:
                        nc.scalar.mul(
                            ut_sb[b][:, p, :], ut_ps[:, dp * 256:(dp + 1) * 256],
                            r_t[b][:, p:p + 1],
                        )

            def emit_w2(b):
                for p in range(4):
                    for cc in (0, 1):
                        nc.tensor.matmul(
                            w2_ps[b][:, cc * 256:(cc + 1) * 256],
                            lhsT=ut_sb[b][:, p, cc * 128:(cc + 1) * 128],
                            rhs=wo_sb[:, p, :],
                            start=(p == 0 and cc == 0), stop=(p == 3),
                            skip_group_check=True,
                        )

            def emit_w2_copies(b):
                for cc in (0, 1):
                    copy2(cc, w2_sb[b][:, cc, :],
                          w2_ps[b][:, cc * 256:(cc + 1) * 256])

            y_group = {}
            Y_GROUPS = {0: [(0, 8), (8, 16), (16, 24), (24, 32)],
                        1: [(0, 8), (8, 14), (14, 20), (20, 24), (24, 28),
                            (28, 30), (30, 32)]}

            def emit_y(b, j0, j1):
                # pair index j covers chunks 2j, 2j+1 in one PSUM bank
                for j in range(j0, j1):
                    k0 = 2 * j
                    gi, (glo, ghi) = next(
                        (i, g) for i, g in enumerate(Y_GROUPS[b])
                        if g[0] <= k0 < g[1]
                    )
                    if k0 == glo:
                        y_group[(b, gi)] = y_sb_pool.tile(
                            [128, ghi - glo, C], F16, name=f"y{b}{gi}", tag="ysb"
                        )
                    pool = big_pool if j % 2 == 0 else g_pool
                    y_ps = pool.tile([128, 512], F32, name="yps",
                                     tag="big" if j % 2 == 0 else "g")
                    for dk in (0, 1):
                        k = k0 + dk
                        for ci in (0, 1):
                            nc.tensor.matmul(
                                y_ps[:, dk * 256:(dk + 1) * 256],
                                lhsT=xt_t[b][ci][:, k * 128:(k + 1) * 128],
                                rhs=w2_sb[b][:, ci, :],
                                start=(dk == 0 and ci == 0),
                                stop=(dk == 1 and ci == 1),
                                skip_group_check=True,
                            )
                    copy2(j, y_group[(b, gi)][:, k0 - glo:k0 - glo + 2, :], y_ps)
                    if k0 + 2 == ghi:
                        nc.sync.dma_start(
                            out=y_d[b, :, glo:ghi, :],
                            in_=y_group[(b, gi)],
                        )

            def emit_chain(b, filler):
                """Attention tail for batch b. `filler(stage)` emits PE filler
                between chain stages (stage index 0..3)."""
                with tc.high_priority():
                    g10_ps = sim_pool.tile([128, 128], F16, name=f"g10{b}",
                                           tag="sim")
                    emit_g_copies(b, g10_ps)
                filler(0)
                with tc.high_priority():
                    tq_ps = [big_pool.tile([128, 512], F32, name=f"tq{b}{cc}",
                                           tag="big") for cc in (0, 1)]
                    emit_tq(b, tq_ps)
                    emit_tq_copies(b, tq_ps)
                filler(1)
                with tc.high_priority():
                    simt_ps = sim_pool.tile([128, 256], F32, name=f"simt{b}",
                                            tag="sim")
                    emit_simt(b, simt_ps)
                    emit_exp(b, simt_ps)
                    emit_sums(b)
                filler(2)
                with tc.high_priority():
                    e_ps = sim_pool.tile([64, 512], F32, name=f"ep{b}",
                                         tag="sim")
                    emit_eprime(b, e_ps)
                    emit_at_copy(b, e_ps)
                    ut_ps = [atut_pool.tile([128, 512], F32, name=f"ut{b}{i}",
                                            tag="atut") for i in (0, 1)]
                    emit_ut(b, ut_ps[0], 0)
                    emit_ut_copies(b, ut_ps[0], 0)
                    emit_ut(b, ut_ps[1], 1)
                    emit_ut_copies(b, ut_ps[1], 1)
                filler(3)
                with tc.high_priority():
                    w2_ps[b] = g_pool.tile([128, 512], F32, name=f"w2ps{b}",
                                           tag="g")
                    emit_w2(b)
                    emit_w2_copies(b)

            # ---------------- program ----------------
            # PE warm-up: tiny fp16 matmuls ramp the p-state while the first
            # xN groups are in flight (results unused; src is a fast memset)
            warm_src = consts.tile([128, 64], F16, name="warmsrc")
            with tc.high_priority():
                nc.gpsimd.memset(warm_src, 0.0)
            warm_ps = g_pool.tile([128, 512], F32, name="warm", tag="g")
            for i in range(44):
                nc.tensor.matmul(
                    warm_ps[0:64, 0:64],
                    lhsT=warm_src[:, 0:64], rhs=warm_src[:, 0:64],
                    start=True, stop=True,
                    skip_group_check=True,
                )

            # DMA order on SP: xn0, weights, xn1, xt0, xt1, (y outs inline)
            emit_xn_dma(0)
            for ci in (0, 1):
                nc.sync.dma_start(out=wqk_sb[ci], in_=wqk_d[ci, :, :])
            nc.sync.dma_start(out=wvt_sb, in_=wvt_d[:, :, :])
            nc.sync.dma_start(out=wo_sb, in_=wo_d[:, :, :])
            emit_g(0, 0, NK)
            emit_xn_dma(1)
            emit_xt_dma(0)

            # batch0 chain with G1 segments as filler
            G1_SEG = [(0, 4), (4, 8), (8, 12), (12, 16)]
            def fill0(stage):
                lo, hi = G1_SEG[stage]
                emit_g(1, lo, hi)
            emit_chain(0, fill0)
            emit_g(1, 16, NK)
            emit_xt_dma(1)

            # batch1 chain with y0 pairs as filler
            Y0_SEG = [(0, 2), (2, 4), (4, 7), (7, 10)]
            def fill1(stage):
                lo, hi = Y0_SEG[stage]
                emit_y(0, lo, hi)
            emit_chain(1, fill1)
            emit_y(0, 10, 16)
            emit_y(1, 0, 16)
    return _split_multi_waits(nc)


def _get_nc():
    if "nc" not in _CACHE:
        _CACHE["nc"] = _build()
    return _CACHE["nc"]


def kernel(x, w_qkv, w_out, b_out, **kw):
    x = np.asarray(x, dtype=np.float32)
    w_qkv = np.asarray(w_qkv, dtype=np.float32)
    w_out = np.asarray(w_out, dtype=np.float32)
    b_out = np.asarray(b_out, dtype=np.float32)

    x2 = x.reshape(BATCH, D, C).astype(np.float16)
    # natural layout chunks: xN[b, p, k, c] = x[b, k*128+p, c]
    xn_all = np.ascontiguousarray(
        x2.reshape(BATCH, NK, 128, C).transpose(0, 2, 1, 3)
    )
    # transposed layout: xT[b, ci, p, d] = x[b, d, ci*128+p]
    xt_all = np.ascontiguousarray(
        x2.transpose(0, 2, 1).reshape(BATCH, 2, 128, D)
    )

    wq = w_qkv[:, :HID] * np.float32(DH ** (-0.5))
    wk = w_qkv[:, HID:2 * HID]
    wqk = np.ascontiguousarray(
        np.concatenate([wq, wk], axis=1).reshape(2, 128, 1024).astype(np.float16)
    )
    # wvt[j, h, c] = w_v[c, h*64+j]
    wvt = np.ascontiguousarray(
        w_qkv[:, 2 * HID:3 * HID].T.reshape(HEADS, 64, C)
        .transpose(1, 0, 2).astype(np.float16)
    )
    wo = np.ascontiguousarray(
        w_out.reshape(4, 128, C).transpose(1, 0, 2).astype(np.float16)
    )

    in_maps = []
    for core in range(N_CORES):
        sl = slice(core * BPC, (core + 1) * BPC)
        in_maps.append({
            "xN": xn_all[sl], "xT": xt_all[sl],
            "wqk": wqk, "wvt": wvt, "wo": wo,
        })

    nc = _get_nc()
    res = run_bass_kernel_spmd(nc, in_maps, core_ids=list(range(N_CORES)), **kw)
    # y_d[b, p, k, c] = y[b, k*128+p, c]
    y = np.concatenate(
        [r["y"].transpose(0, 2, 1, 3).reshape(BPC, D, C) for r in res.results],
        axis=0,
    ).astype(np.float32)
    y += b_out
    return y.reshape(BATCH, 64, 64, C)


# revision 38
# speedup vs baseline: 3.7287x; 1.0769x over previous
"""Channel-attention (per-head [64,64] score matrix) Trainium2 Bass kernel.

Algebraic restructuring vs the direct q/k/v formulation: since the score
matrix contracts the full spatial axis, attention only needs the Gram matrix
    G = x^T x                        # [256,256]; symmetric: 3 quarter-blocks
    sim_h = (w_q_h/8)^T G w_k_h      # via t = G @ w_k (G10 = G01^T by PE
    attn_h = softmax(sim_h)          #  transpose), all heads packed
    W2 = sum_h w_v_h attn_h^T w_out_h    # [256,256] fused output operator
    y = x @ W2 (+ b_out on host)
~620M MACs/batch vs 2.4G for the direct path (~4x less PE work).

Distribution: data-parallel over batch - 8 cores x 2 batches each, weights
replicated, no collectives. Host sends x in BOTH layouts (natural [d,C] for
G, transposed [C,d] for y) since the PE only contracts the partition dim;
fp16 operands everywhere with fp32 PSUM accumulation; y returns fp16 and the
bias-add/upcast happen on the host.

Softmax path (per batch): fused row-max (negated) -> one broadcast add of
the shift -> PE-transpose of the SHIFTED logits -> exp directly drains the
transposed PSUM into the fp16 uT stationary (no extra copy); a second exp in
the i-layout feeds the row sums, and 1/s is folded into the uT PSUM drain as
a per-partition scale.

Schedule: two batches pipelined; batch0's chain hides under batch1's Gram
matmuls and batch1's chain hides under batch0's y pairs (chain stages are
high_priority so the scheduler threads them through the bulk work). y output
chunks go out in grouped DMAs with a finely split tail so the final transfer
(which gates the drain) is short. PE warm-up matmuls ramp the clock p-state
while the first DMAs are in flight. PSUM accumulators that share a bank rely
on in-order start=True bank-zeroing or explicit first-write zeroing.
"""

import numpy as np

import concourse.bass as bass
import concourse.mybir as mybir
from concourse.bass_utils import run_bass_kernel_spmd
from concourse.masks import make_identity
from concourse.tile import TileContext


def _split_multi_waits(nc, limit=1):
    """Post-pass: the walrus build in this container rejects instructions
    carrying more than `limit` sync-waits ("Too many sync wait commands" in
    setupSyncWait). Tile attaches up to 3. Hoist the extras onto same-engine
    NoOp instructions inserted immediately before the owner - the engine
    sequencer executes them in order, so the ordering semantics are
    identical."""
    drain_engines = [
        mybir.EngineType.PE,
        mybir.EngineType.DVE,
        mybir.EngineType.Activation,
        mybir.EngineType.Pool,
        mybir.EngineType.SP,
    ]
    n_split = 0
    for f in nc.m.functions:
        for blk in f.blocks:
            il = blk.instructions
            i = 0
            while i < len(il):
                inst = il[i]
                si = inst.sync_info
                waits = list(si.on_wait) if si is not None else []
                if len(waits) > limit:
                    si.on_wait = waits[:limit]
                    is_drain = type(inst).__name__ == "InstDrain"
                    for k, w in enumerate(waits[limit:]):
                        nop = mybir.InstNoOp(
                            name=f"I-waitsplit-{n_split}", ins=[], outs=[]
                        )
                        n_split += 1
                        nop.engine = (
                            drain_engines[k % len(drain_engines)]
                            if is_drain else inst.engine
                        )
                        nop.sync_info = mybir.SyncInfo(on_wait=[w], on_update=[])
                        il.insert(i, nop)
                        i += 1
                i += 1
    return nc


N_CORES = 8
BATCH = 16
BPC = BATCH // N_CORES  # batches per core
D = 4096   # spatial (64*64)
C = 256    # channels
HID = 512
HEADS = 8
DH = 64
NK = 32    # d-chunks of 128

F32 = mybir.dt.float32
F16 = mybir.dt.float16
BF16 = mybir.dt.bfloat16

_CACHE = {}


def _build():
    nc = bass.Bass()
    xn_d = nc.declare_dram_parameter("xN", [BPC, 128, NK, C], F16, isOutput=False)
    xt_d = nc.declare_dram_parameter("xT", [BPC, 2, 128, D], F16, isOutput=False)
    wqk_d = nc.declare_dram_parameter("wqk", [2, 128, 1024], F16, isOutput=False)
    wvt_d = nc.declare_dram_parameter("wvt", [64, HEADS, C], F16, isOutput=False)
    wo_d = nc.declare_dram_parameter("wo", [128, 4, C], F16, isOutput=False)
    y_d = nc.declare_dram_parameter("y", [BPC, 128, NK, C], F16, isOutput=True)

    with TileContext(nc) as tc:
        with (
            tc.tile_pool(name="consts", bufs=1) as consts,
            tc.tile_pool(name="xn", bufs=2) as xn_pool,
            tc.tile_pool(name="xt", bufs=4) as xt_pool,
            tc.tile_pool(name="small", bufs=2) as sm_pool,
            tc.tile_pool(name="small4", bufs=4) as sm4_pool,
            tc.tile_pool(name="small6", bufs=6) as sm6_pool,
            tc.tile_pool(name="ysb", bufs=10) as y_sb_pool,
            tc.tile_pool(name="gps", bufs=2, space="PSUM") as g_pool,
            tc.tile_pool(name="big", bufs=3, space="PSUM") as big_pool,
            tc.tile_pool(name="simp", bufs=1, space="PSUM") as sim_pool,
            tc.tile_pool(name="atut", bufs=2, space="PSUM") as atut_pool,
        ):
            # ---- constant tiles ----
            wqk_sb = [consts.tile([128, 1024], F16, name=f"wqk{ci}") for ci in (0, 1)]
            wvt_sb = consts.tile([64, HEADS, C], F16, name="wvt")
            wo_sb = consts.tile([128, 4, C], F16, name="wo")
            ident32 = consts.tile([128, 128], F32, name="ident32")
            make_identity(nc, ident32)
            identh = consts.tile([128, 128], F16, name="identh")
            make_identity(nc, identh)

            # per-batch SBUF tiles
            xn_t = [xn_pool.tile([128, NK, C], F16, name=f"xn{b}", tag="xn")
                    for b in (0, 1)]
            xt_t = [[xt_pool.tile([128, D], F16, name=f"xt{b}{ci}", tag="xt")
                     for ci in (0, 1)] for b in (0, 1)]
            g_sb = [sm_pool.tile([128, 512], F16, name=f"gsb{b}", tag="gsb")
                    for b in (0, 1)]
            tq_sb = [[sm4_pool.tile([128, 512], F16, name=f"tqsb{b}{cc}", tag="tqsb")
                      for cc in (0, 1)] for b in (0, 1)]
            s_t = [sm6_pool.tile([128, 4], F32, name=f"s{b}", tag="stat") for b in (0, 1)]
            r_t = [sm6_pool.tile([128, 4], F32, name=f"r{b}", tag="stat") for b in (0, 1)]
            m_t = [sm6_pool.tile([128, 4], F32, name=f"m{b}", tag="stat")
                   for b in (0, 1)]
            apair = [sm_pool.tile([128, 4, 64], F32, name=f"ap{b}", tag="ap")
                     for b in (0, 1)]
            sadj = [sm_pool.tile([128, 4, 64], F32, name=f"sadj{b}", tag="sadj")
                    for b in (0, 1)]
            at_sb = [sm_pool.tile([64, 4, 128], F16, name=f"at{b}", tag="at")
                     for b in (0, 1)]
            ut_sb = [sm_pool.tile([128, 4, C], F16, name=f"ut{b}", tag="ut")
                     for b in (0, 1)]
            w2_sb = [sm_pool.tile([128, 2, C], F16, name=f"w2{b}", tag="w2")
                     for b in (0, 1)]

            g_ps = [g_pool.tile([128, 512], F32, name=f"gps{b}", tag="g")
                    for b in (0, 1)]
            w2_ps = [None, None]

            def copy2(i, out, in_):
                # alternate DVE tensor_copy / ACT activation-copy
                if i % 2 == 0:
                    nc.vector.tensor_copy(out, in_)
                else:
                    nc.scalar.copy(out, in_)

            # ---------------- emission helpers ----------------
            def emit_xn_dma(b):
                lo = 0
                for gsz in (2, 3, 4, 4, 4, 5, 5, 5):
                    hi = lo + gsz
                    nc.sync.dma_start(
                        out=xn_t[b][:, lo:hi, :], in_=xn_d[b, :, lo:hi, :]
                    )
                    lo = hi

            def emit_xt_dma(b):
                for ci in (0, 1):
                    for half in (0, 1):
                        nc.sync.dma_start(
                            out=xt_t[b][ci][:, half * 2048:(half + 1) * 2048],
                            in_=xt_d[b, ci, :, half * 2048:(half + 1) * 2048],
                        )

            def emit_g(b, k0, k1):
                # G symmetric: only G00, G01, G11 accumulate (G10 = G01^T is
                # reconstructed by a PE transpose afterwards). First write
                # zeroes the whole bank.
                for k in range(k0, k1):
                    for qi, (ca, cb) in enumerate(((0, 0), (0, 1), (1, 1))):
                        nc.tensor.matmul(
                            g_ps[b][:, qi * 128:(qi + 1) * 128],
                            lhsT=xn_t[b][:, k, ca * 128:(ca + 1) * 128],
                            rhs=xn_t[b][:, k, cb * 128:(cb + 1) * 128],
                            start=(k == 0 and qi == 0),
                            stop=(k == NK - 1 and qi == 2),
                            skip_group_check=True,
                        )

            def emit_g_copies(b, g10_ps):
                # drain the three computed quarters, then rebuild G10 = G01^T
                # on the PE (consumed last by the tq matmuls)
                copy2(0, g_sb[b][:, 128:256], g_ps[b][:, 128:256])
                copy2(1, g_sb[b][:, 0:128], g_ps[b][:, 0:128])
                nc.tensor.matmul(
                    g10_ps, lhsT=g_sb[b][:, 128:256], rhs=identh,
                    is_transpose=True, start=True, stop=True,
                    skip_group_check=True,
                )
                copy2(1, g_sb[b][:, 256:384], g_ps[b][:, 256:384])
                copy2(0, g_sb[b][:, 384:512], g10_ps)

            TQ_Q = {(0, 0): 0, (0, 1): 3, (1, 0): 1, (1, 1): 2}

            def emit_tq(b, tq_ps):
                # t = G @ w_k; order so the reconstructed Q10 is needed last
                for cc in (1, 0):
                    for ci2 in (0, 1):
                        q = TQ_Q[(cc, ci2)]
                        nc.tensor.matmul(
                            tq_ps[cc],
                            lhsT=g_sb[b][:, q * 128:(q + 1) * 128],
                            rhs=wqk_sb[ci2][:, 512:1024],
                            start=(ci2 == 0), stop=(ci2 == 1),
                        )

            def emit_tq_copies(b, tq_ps):
                for cc in (0, 1):
                    copy2(cc, tq_sb[b][cc], tq_ps[cc])

            def emit_simt(b, simt_ps):
                # sim_h[i, j] = w_q_h^T (G w_k_h): pair p in col block p,
                # head parity in row halves (i on partitions for the softmax)
                for h in range(HEADS):
                    p, par = h // 2, h % 2
                    for cc in (0, 1):
                        nc.tensor.matmul(
                            simt_ps[par * 64:par * 64 + 64, p * 64:(p + 1) * 64],
                            lhsT=wqk_sb[cc][:, h * 64:h * 64 + 64],
                            rhs=tq_sb[b][cc][:, h * 64:h * 64 + 64],
                            start=(h < 2 and cc == 0), stop=(cc == 1),
                            skip_group_check=True,
                        )

            def emit_exp(b, simt_ps):
                # shift logits: row-max (negated) + one broadcast add
                nc.vector.reduce_max(
                    out=m_t[b][:, :],
                    in_=simt_ps[:, :].rearrange("p (a b) -> p a b", a=4),
                    axis=mybir.AxisListType.X,
                    negate=True,
                )
                nc.vector.tensor_add(
                    sadj[b][:, :, :],
                    simt_ps[:, :].rearrange("p (a b) -> p a b", a=4),
                    m_t[b][:, :].broadcast_to([128, 4, 64]),
                )

            def emit_eprime(b, e_ps):
                # transpose the shifted logits to [j, (p, i-stacked)]
                for p in range(4):
                    nc.tensor.matmul(
                        e_ps[:, p * 128:(p + 1) * 128],
                        lhsT=sadj[b][:, p, :],
                        rhs=ident32,
                        is_transpose=True,
                        start=(p == 0), stop=(p == 3),
                        skip_group_check=True,
                    )

            def emit_at_copy(b, e_ps):
                # exp doubles as the PSUM drain: writes e^T straight to fp16
                nc.scalar.activation(
                    out=at_sb[b][:, :, :],
                    in_=e_ps[:, :],
                    func=mybir.ActivationFunctionType.Exp,
                    scale=1.0,
                )

            def emit_sums(b):
                # off-critical: row sums from a second exp in the i-layout
                nc.scalar.activation(
                    out=apair[b][:, :, :],
                    in_=sadj[b][:, :, :],
                    func=mybir.ActivationFunctionType.Exp,
                    scale=1.0,
                )
                nc.vector.reduce_sum(
                    out=s_t[b][:, :],
                    in_=apair[b][:, :, :],
                    axis=mybir.AxisListType.X,
                )
                nc.vector.reciprocal(r_t[b], s_t[b])

            def emit_ut(b, ut_ps, pp):
                # raw (unnormalized) uT: lhsT = e_h^T slice, rhs = w_v^T rows
                for dp in (0, 1):
                    p = 2 * pp + dp
                    for par in (0, 1):
                        h = 2 * p + par
                        nc.tensor.matmul(
                            ut_ps[par * 64:par * 64 + 64, dp * 256:(dp + 1) * 256],
                            lhsT=at_sb[b][:, p, par * 64:par * 64 + 64],
                            rhs=wvt_sb[:, h, :],
                            start=(dp == 0), stop=(dp == 1),
                            skip_group_check=True,
                        )

            def emit_ut_copies(b, ut_ps, pp):
                # fold the softmax normalizer in during the PSUM drain
                for dp in (0, 1):
                    p = 2 * pp + dp
                    if dp == 0:
                        nc.vector.tensor_scalar_mul(
                            ut_sb[b][:, p, :], ut_ps[:, dp * 256:(dp + 1) * 256],
                            r_t[b][:, p:p + 1],
                        )
                    else:
                        nc.scalar.mul(
                            ut_sb[b][:, p, :], ut_ps[:, dp * 256:(dp + 1) * 256],
                            r_t[b][:, p:p + 1],
                        )

            def emit_w2(b):
                for p in range(4):
                    for cc in (0, 1):
                        nc.tensor.matmul(
                            w2_ps[b][:, cc * 256:(cc + 1) * 256],
                            lhsT=ut_sb[b][:, p, cc * 128:(cc + 1) * 128],
                            rhs=wo_sb[:, p, :],
                            start=(p == 0 and cc == 0), stop=(p == 3),
                            skip_group_check=True,
                        )

            def emit_w2_copies(b):
                for cc in (0, 1):
                    copy2(cc, w2_sb[b][:, cc, :],
                          w2_ps[b][:, cc * 256:(cc + 1) * 256])

            y_group = {}
            Y_GROUPS = {0: [(0, 8), (8, 16), (16, 24), (24, 32)],
                        1: [(0, 8), (8, 14), (14, 20), (20, 24), (24, 28),
                            (28, 30), (30, 32)]}

            def emit_y(b, j0, j1):
                # pair index j covers chunks 2j, 2j+1 in one PSUM bank
                for j in range(j0, j1):
                    k0 = 2 * j
                    gi, (glo, ghi) = next(
                        (i, g) for i, g in enumerate(Y_GROUPS[b])
                        if g[0] <= k0 < g[1]
                    )
                    if k0 == glo:
                        y_group[(b, gi)] = y_sb_pool.tile(
                            [128, ghi - glo, C], F16, name=f"y{b}{gi}", tag="ysb"
                        )
                    pool = big_pool if j % 2 == 0 else g_pool
                    y_ps = pool.tile([128, 512], F32, name="yps",
                                     tag="big" if j % 2 == 0 else "g")
                    for dk in (0, 1):
                        k = k0 + dk
                        for ci in (0, 1):
                            nc.tensor.matmul(
                                y_ps[:, dk * 256:(dk + 1) * 256],
                                lhsT=xt_t[b][ci][:, k * 128:(k + 1) * 128],
                                rhs=w2_sb[b][:, ci, :],
                                start=(dk == 0 and ci == 0),
                                stop=(dk == 1 and ci == 1),
                                skip_group_check=True,
                            )
                    copy2(j, y_group[(b, gi)][:, k0 - glo:k0 - glo + 2, :], y_ps)
                    if k0 + 2 == ghi:
                        nc.sync.dma_start(
                            out=y_d[b, :, glo:ghi, :],
                            in_=y_group[(b, gi)],
                        )

            def emit_chain(b, filler):
                """Attention tail for batch b. `filler(stage)` emits PE filler
                between chain stages (stage index 0..3)."""
                with tc.high_priority():
                    g10_ps = sim_pool.tile([128, 128], F16, name=f"g10{b}",
                                           tag="sim")
                    emit_g_copies(b, g10_ps)
                filler(0)
                with tc.high_priority():
                    tq_ps = [big_pool.tile([128, 512], F32, name=f"tq{b}{cc}",
                                           tag="big") for cc in (0, 1)]
                    emit_tq(b, tq_ps)
                    emit_tq_copies(b, tq_ps)
                filler(1)
                with tc.high_priority():
                    simt_ps = sim_pool.tile([128, 256], F32, name=f"simt{b}",
                                            tag="sim")
                    emit_simt(b, simt_ps)
                    emit_exp(b, simt_ps)
                filler(2)
                with tc.high_priority():
                    e_ps = sim_pool.tile([64, 512], F32, name=f"ep{b}",
                                         tag="sim")
                    emit_eprime(b, e_ps)
                    emit_sums(b)
                    emit_at_copy(b, e_ps)
                    ut_ps = [atut_pool.tile([128, 512], F32, name=f"ut{b}{i}",
                                            tag="atut") for i in (0, 1)]
                    emit_ut(b, ut_ps[0], 0)
                    emit_ut_copies(b, ut_ps[0], 0)
                    emit_ut(b, ut_ps[1], 1)
                    emit_ut_copies(b, ut_ps[1], 1)
                filler(3)
                with tc.high_priority():
                    w2_ps[b] = g_pool.tile([128, 512], F32, name=f"w2ps{b}",
                                           tag="g")
                    emit_w2(b)
                    emit_w2_copies(b)

            # ---------------- program ----------------
            # PE warm-up: tiny fp16 matmuls ramp the p-state while the first
            # xN groups are in flight (results unused; src is a fast memset)
            warm_src = consts.tile([128, 64], F16, name="warmsrc")
            with tc.high_priority():
                nc.gpsimd.memset(warm_src, 0.0)
            warm_ps = g_pool.tile([128, 512], F32, name="warm", tag="g")
            for i in range(28):
                nc.tensor.matmul(
                    warm_ps[0:64, 0:64],
                    lhsT=warm_src[:, 0:64], rhs=warm_src[:, 0:64],
                    start=True, stop=True,
                    skip_group_check=True,
                )

            # DMA order on SP: xn0, weights, xn1, xt0, xt1, (y outs inline)
            emit_xn_dma(0)
            for ci in (0, 1):
                nc.sync.dma_start(out=wqk_sb[ci], in_=wqk_d[ci, :, :])
            nc.sync.dma_start(out=wvt_sb, in_=wvt_d[:, :, :])
            nc.sync.dma_start(out=wo_sb, in_=wo_d[:, :, :])
            emit_g(0, 0, NK)
            emit_xn_dma(1)
            emit_xt_dma(0)

            # batch0 chain with G1 segments as filler
            G1_SEG = [(0, 4), (4, 8), (8, 12), (12, 16)]
            def fill0(stage):
                lo, hi = G1_SEG[stage]
                emit_g(1, lo, hi)
            emit_chain(0, fill0)
            emit_g(1, 16, NK)
            emit_xt_dma(1)

            # batch1 chain with y0 pairs as filler
            Y0_SEG = [(0, 2), (2, 4), (4, 7), (7, 10)]
            def fill1(stage):
                lo, hi = Y0_SEG[stage]
                emit_y(0, lo, hi)
            emit_chain(1, fill1)
            emit_y(0, 10, 16)
            emit_y(1, 0, 16)
    return _split_multi_waits(nc)


def _get_nc():
    if "nc" not in _CACHE:
        _CACHE["nc"] = _build()
    return _CACHE["nc"]


def kernel(x, w_qkv, w_out, b_out, **kw):
    x = np.asarray(x, dtype=np.float32)
    w_qkv = np.asarray(w_qkv, dtype=np.float32)
    w_out = np.asarray(w_out, dtype=np.float32)
    b_out = np.asarray(b_out, dtype=np.float32)

    x2 = x.reshape(BATCH, D, C).astype(np.float16)
    # natural layout chunks: xN[b, p, k, c] = x[b, k*128+p, c]
    xn_all = np.ascontiguousarray(
        x2.reshape(BATCH, NK, 128, C).transpose(0, 2, 1, 3)
    )
    # transposed layout: xT[b, ci, p, d] = x[b, d, ci*128+p]
    xt_all = np.ascontiguousarray(
        x2.transpose(0, 2, 1).reshape(BATCH, 2, 128, D)
    )

    wq = w_qkv[:, :HID] * np.float32(DH ** (-0.5))
    wk = w_qkv[:, HID:2 * HID]
    wqk = np.ascontiguousarray(
        np.concatenate([wq, wk], axis=1).reshape(2, 128, 1024).astype(np.float16)
    )
    # wvt[j, h, c] = w_v[c, h*64+j]
    wvt = np.ascontiguousarray(
        w_qkv[:, 2 * HID:3 * HID].T.reshape(HEADS, 64, C)
        .transpose(1, 0, 2).astype(np.float16)
    )
    wo = np.ascontiguousarray(
        w_out.reshape(4, 128, C).transpose(1, 0, 2).astype(np.float16)
    )

    in_maps = []
    for core in range(N_CORES):
        sl = slice(core * BPC, (core + 1) * BPC)
        in_maps.append({
            "xN": xn_all[sl], "xT": xt_all[sl],
            "wqk": wqk, "wvt": wvt, "wo": wo,
        })

    nc = _get_nc()
    res = run_bass_kernel_spmd(nc, in_maps, core_ids=list(range(N_CORES)), **kw)
    # y_d[b, p, k, c] = y[b, k*128+p, c]
    y = np.concatenate(
        [r["y"].transpose(0, 2, 1, 3).reshape(BPC, D, C) for r in res.results],
        axis=0,
    ).astype(np.float32)
    y += b_out
    return y.reshape(BATCH, 64, 64, C)
